# revision 29
# baseline (speedup 1.0000x reference)
"""Depthwise causal Conv1D (B=4, C=4096, L=4096, K=4) on 8 trn2 NeuronCores.

Sharding: channel-parallel (tensor parallel) — core i owns channels
[i*512, (i+1)*512). Depthwise conv has zero cross-channel interaction, so
there is no communication; each core computes its channel slab end to end.

Per-core kernel layout: channels on SBUF partitions (128 at a time), time on
the free dim. The 4-tap causal FIR along the free dim is computed as four
shifted multiply-accumulate passes with per-partition (per-channel) scalar
weights, split across three engines so no single engine is the bottleneck:

  ScalarE : out[3:L+3]  = w0 * x + bias   (activation, per-partition scale+bias)
            out[0:3]    = bias
  GPSIMD  : out[0:L]   += w3 * x          (scalar_tensor_tensor)
  VectorE : out[1:L+1] += w2 * x          (scalar_tensor_tensor)
            out[2:L+2] += w1 * x          (scalar_tensor_tensor)

DMA (HWDGE via nc.sync) streams 128x4096 fp32 tiles in and 128x4099 tiles
out; the kernel is HBM-bandwidth bound (~64 MB per core total traffic).
"""

import numpy as np

import concourse.bass as bass
import concourse.tile as tile
from concourse import bacc, mybir
from concourse.bass_utils import run_bass_kernel_spmd

B, C, L, K = 4, 4096, 4096, 4
PAD = K - 1
LOUT = L + PAD  # 4099
NCORES = 8
CS = C // NCORES  # 512 channels per core
DT = mybir.dt.float32

_AF = mybir.ActivationFunctionType
_OP = mybir.AluOpType


def build_nc(b=B, cs=CS, l=L, k=K, n_bufs=5, n_edge_chunks=4, pe_cols=2048):
    """Build the per-core Bass program. Parameterized for small-size sim tests.

    Channels on partitions, time on the free dim. x is loaded into a
    [128, pad + l + pad] tile with `pad` zero columns at both ends
    (xp[i] = x[i - pad]), so every tap reads in-bounds and the causal
    zero-padding falls out of the zero stuffing.

    Work split per tile:
      ScalarE : out[pad:lout] = w0 * xp[pad:lout] + bias; head cols = bias
      PE      : taps 1..k-1 for out cols [0, pe_cols) via diagonal weight
                matmuls accumulating in PSUM (out[m] += sum_t w_t*xp[m+t]),
                512-col chunks, fp32
      VectorE : PSUM chunks merged into out (tensor_tensor add), and
                taps 1..k-1 for out cols [pe_cols, lout) via fused
                scalar_tensor_tensor ops
    Stores issue from ScalarE's HWDGE, deferred one iteration; loads from
    SP. GpSimd stays idle (its tensor ops serialize against VectorE on the
    shared SBUF port pair).

    The first and last tiles are split column-wise into `n_edge_chunks`
    pieces (DVE-only taps) to shorten the pipeline ramp and drain.
    """
    ng = cs // 128
    pad = k - 1
    lout = l + pad
    wx = l + 2 * pad  # padded x width
    assert pe_cols % 512 == 0 and pe_cols + pad <= l

    nc = bacc.Bacc("TRN2", target_bir_lowering=False, debug=False, num_devices=NCORES)
    x_d = nc.dram_tensor("x", [b, cs, l], DT, kind="ExternalInput").ap()
    # packed per-channel constants: wb[c] = [w_0..w_{k-1}, bias]
    wb_d = nc.dram_tensor("wb", [cs, k + 1], DT, kind="ExternalInput").ap()
    eye_d = nc.dram_tensor("eye", [128, 128], DT, kind="ExternalInput").ap()
    o_d = nc.dram_tensor("out", [b, cs, lout], DT, kind="ExternalOutput").ap()

    with tile.TileContext(nc) as tc:
        with (
            tc.tile_pool(name="consts", bufs=1) as cpool,
            tc.tile_pool(name="xs", bufs=n_bufs) as xpool,
            tc.tile_pool(name="os", bufs=n_bufs) as opool,
            tc.tile_pool(name="ps", bufs=8, space="PSUM") as ppool,
        ):
            # Constants are emitted lazily (after the first x chunk load) so
            # the first compute tile's data leads the SP DMA trigger queue.
            consts = []
            diags = {}

            def emit_consts():
                # Per-group constant columns: [128, k+1] = w_0..w_{k-1}, bias.
                for g in range(ng):
                    ct = cpool.tile([128, k + 1], DT, tag=f"c{g}")
                    nc.sync.dma_start(ct[:], wb_d[g * 128 : (g + 1) * 128, :])
                    consts.append(ct)
                # identity and per-(group, tap) diagonal weight matrices for PE
                if pe_cols > 0:
                    ident = cpool.tile([128, 128], DT, tag="eye")
                    nc.sync.dma_start(ident[:], eye_d[:])
                    for g in range(ng):
                        for t in range(1, k):
                            dg = cpool.tile([128, 128], DT, tag=f"d{g}_{t}")
                            nc.vector.tensor_scalar(
                                out=dg[:], in0=ident[:],
                                scalar1=consts[g][:, t : t + 1],
                                scalar2=None, op0=_OP.mult,
                            )
                            diags[(g, t)] = dg

            n_tiles = b * ng
            pending_stores = []  # deferred to keep ACT's HWDGE queue unblocked

            def flush_stores():
                for dst, src in pending_stores:
                    nc.scalar.dma_start(dst, src)
                pending_stores.clear()

            ti = 0
            for bi in range(b):
                for g in range(ng):
                    c0 = g * 128
                    first, last = ti == 0, ti == n_tiles - 1
                    edge = first or last
                    nchunk = n_edge_chunks if edge else 1
                    cw = l // nchunk
                    n_pe = 0 if edge else pe_cols  # edge tiles are DVE-only

                    xt = xpool.tile([128, wx], DT, tag="x")
                    # zero stuffing: xp[0:pad] = xp[pad+l:] = 0 (GpSimd: tiny,
                    # keeps the VectorE queue free of slot-recycle waits)
                    nc.gpsimd.memset(xt[:, 0:pad], 0.0)
                    nc.gpsimd.memset(xt[:, pad + l : wx], 0.0)
                    if first:
                        # chunk 0 load leads the SP queue; consts follow it
                        nc.sync.dma_start(
                            xt[:, pad : pad + cw], x_d[bi, c0 : c0 + 128, 0:cw]
                        )
                        emit_consts()
                        for c in range(1, nchunk):
                            nc.sync.dma_start(
                                xt[:, pad + c * cw : pad + (c + 1) * cw],
                                x_d[bi, c0 : c0 + 128, c * cw : (c + 1) * cw],
                            )
                    else:
                        nc.sync.dma_start(
                            xt[:, pad : pad + l], x_d[bi, c0 : c0 + 128, :]
                        )
                    ot = opool.tile([128, lout], DT, tag="o")
                    ct = consts[g]

                    for c in range(nchunk):
                        j0, j1 = c * cw, (c + 1) * cw
                        # tap 0 (+bias): out[pad+j] = w0*x[j] + bias  (ScalarE)
                        nc.scalar.activation(
                            ot[:, pad + j0 : pad + j1],
                            xt[:, pad + j0 : pad + j1], _AF.Identity,
                            bias=ct[:, k : k + 1], scale=ct[:, 0:1],
                        )
                        if c == 0:
                            # head columns [0:pad] = bias  (ScalarE)
                            nc.scalar.activation(
                                ot[:, 0:pad], xt[:, 0:pad], _AF.Identity,
                                bias=ct[:, k : k + 1], scale=0.0,
                            )
                            flush_stores()
                        # PE portion: out[m] += sum_t w_t * xp[m+t], m in [0, n_pe)
                        if c == 0 and n_pe > 0:
                            for m0 in range(0, n_pe, 512):
                                pt = ppool.tile([128, 512], DT, tag="p")
                                for t in range(1, k):
                                    nc.tensor.matmul(
                                        pt[:], lhsT=diags[(g, t)][:],
                                        rhs=xt[:, m0 + t : m0 + t + 512],
                                        start=(t == 1), stop=(t == k - 1),
                                    )
                                nc.vector.tensor_tensor(
                                    out=ot[:, m0 : m0 + 512],
                                    in0=pt[:], in1=ot[:, m0 : m0 + 512], op=_OP.add,
                                )
                        # DVE taps: out[m] += w_t * xp[m+t].
                        # On edge tiles, chunk c handles out [j0-pad, j1-pad)
                        # so its tap reads stay within x chunks <= c (xp idx
                        # m+t <= j1-1), keeping the ramp free of forward deps.
                        if edge:
                            m_lo = 0 if c == 0 else j0 - pad
                            m_hi = lout if c == nchunk - 1 else j1 - pad
                        else:
                            m_lo = max(j0, n_pe)
                            m_hi = lout if c == nchunk - 1 else j1
                        if m_hi > m_lo:
                            for t in range(k - 1, 0, -1):
                                nc.vector.scalar_tensor_tensor(
                                    out=ot[:, m_lo:m_hi],
                                    in0=xt[:, m_lo + t : m_hi + t],
                                    scalar=ct[:, t : t + 1],
                                    in1=ot[:, m_lo:m_hi],
                                    op0=_OP.mult, op1=_OP.add,
                                )
                        if last:
                            # store exactly the finalized range of this chunk
                            nc.scalar.dma_start(
                                o_d[bi, c0 : c0 + 128, m_lo:m_hi], ot[:, m_lo:m_hi]
                            )
                    if not last:
                        pending_stores.append((o_d[bi, c0 : c0 + 128, :], ot[:]))
                    ti += 1
            flush_stores()
    nc.compile()
    return nc


_cached_nc = None


def _get_nc():
    global _cached_nc
    if _cached_nc is None:
        _cached_nc = build_nc()
    return _cached_nc


def run(x, kernel, bias, trace=False, **kwargs):
    """Shard, run on 8 cores, gather. Returns (out, BassKernelResults)."""
    x = np.ascontiguousarray(x, dtype=np.float32)
    w = np.asarray(kernel, dtype=np.float32).reshape(K, C)
    bvec = np.asarray(bias, dtype=np.float32).reshape(C)
    # wb[c] = [w_0[c] .. w_{K-1}[c], bias[c]]
    wb = np.concatenate([w.T, bvec[:, None]], axis=1).astype(np.float32)

    eye = np.eye(128, dtype=np.float32)
    in_maps = []
    for i in range(NCORES):
        sl = slice(i * CS, (i + 1) * CS)
        in_maps.append(
            {
                "x": np.ascontiguousarray(x[:, sl, :]),
                "wb": np.ascontiguousarray(wb[sl, :]),
                "eye": eye,
            }
        )

    nc = _get_nc()
    bkr = run_bass_kernel_spmd(
        nc, in_maps, core_ids=list(range(NCORES)), trace=trace, **kwargs
    )
    out = np.concatenate([r["out"] for r in bkr.results], axis=1)
    return out, bkr


def kernel(x, kernel, bias):
    import os

    prev = os.environ.get("BASS_NEVER_TRACE")
    os.environ["BASS_NEVER_TRACE"] = "1"  # keep the runner off the NTFF path
    try:
        out, _ = run(x, kernel, bias)
    finally:
        if prev is None:
            os.environ.pop("BASS_NEVER_TRACE", None)
        else:
            os.environ["BASS_NEVER_TRACE"] = prev
    return out
